# revision 18
# baseline (speedup 1.0000x reference)
"""BitLinear (RMSNorm + ternary-quantized linear) on 8 TRN2 NeuronCores.

Sharding: data-parallel over tokens (B*S = 8192 -> 1024 per core), weight
replicated. gamma = mean(|w|) is computed exactly via per-core partial sums
over a 256-row slice of the weight (passed as the extra sharded input
"wslice") + a tiny AllReduce, so all cores quantize with the identical gamma.

Math per core:
  xn   = x / sqrt(mean(x^2) + 1e-6) * norm_weight        (f32 stats, bf16 out)
  w_q  = sign(w) * (|w| >= 0.5*(gamma + 1e-8))           ({-1,0,1}, exact bf16)
  out  = (xn @ w_q^T) * gamma                            (bf16 matmul, f32 out)

The threshold form equals clip(round(w/(gamma+eps)), -1, 1) because
max|w| < 1.5*gamma for this weight distribution (and values rounding to +-2
clip back to +-1 anyway).
"""

import os
import sys

for _p in ("/opt/trn_rl_repo",):
    if _p not in sys.path:
        sys.path.insert(0, _p)

import numpy as np

import concourse.bass as bass
import concourse.bacc as bacc
import concourse.tile as tile
import concourse.mybir as mybir
from concourse import masks
from concourse.bass_utils import run_bass_kernel_spmd

NORM_EPS = 1e-6
QUANT_EPS = 1e-8

B, S, DIN, DOUT = 2, 4096, 2048, 2048
NCORES = 8
TOKS = B * S              # 8192 total tokens
TOK = TOKS // NCORES      # 1024 tokens per core
TT = TOK // 128           # 8 token tiles per core
KC = DIN // 128           # 16 contraction chunks
NB = DOUT // 512          # 4 output column blocks
WS = DOUT // NCORES       # 256 weight rows per core for the gamma partial
CCPAD = 512               # AllReduce payload padded to 2KB

F32 = mybir.dt.float32
BF16 = mybir.dt.bfloat16
ALU = mybir.AluOpType
ACTF = mybir.ActivationFunctionType


def _build():
    nc = bacc.Bacc(
        "TRN2", target_bir_lowering=False, debug=False, num_devices=NCORES
    )

    x_d = nc.dram_tensor("x", [TOK, DIN], F32, kind="ExternalInput")
    w_d = nc.dram_tensor("weight", [DOUT, DIN], F32, kind="ExternalInput")
    ws_d = nc.dram_tensor("wslice", [WS, DIN], F32, kind="ExternalInput")
    nw_d = nc.dram_tensor("norm_weight", [DIN], F32, kind="ExternalInput")
    out_d = nc.dram_tensor("out", [TOK, DOUT], F32, kind="ExternalOutput")

    with tile.TileContext(nc) as tc:
        with (
            tc.tile_pool(name="const", bufs=1) as const,
            tc.tile_pool(name="dramp", bufs=1, space="DRAM") as dramp,
            tc.tile_pool(name="gpool", bufs=1) as gpool,
            tc.tile_pool(name="spool", bufs=4) as spool,
            tc.tile_pool(name="xin", bufs=2) as xin,
            tc.tile_pool(name="xnp", bufs=2) as xnp,
            tc.tile_pool(name="xntp", bufs=TT) as xntp,
            tc.tile_pool(name="wf", bufs=3) as wf,
            tc.tile_pool(name="wm", bufs=2) as wm,
            tc.tile_pool(name="wqp", bufs=2) as wqp,
            tc.tile_pool(name="osb", bufs=3) as osb,
            tc.tile_pool(name="psg", bufs=1, space="PSUM") as psg,
            tc.tile_pool(name="pst", bufs=4, space="PSUM") as pst,
            tc.tile_pool(name="pso", bufs=3, space="PSUM") as pso,
        ):
            # ---- constants ----
            ident = const.tile([128, 128], BF16)
            masks.make_identity(nc, ident[:])
            ones = const.tile([128, 128], F32)
            nc.gpsimd.memset(ones[:], 1.0)
            eps_sb = const.tile([128, 1], F32)
            nc.gpsimd.memset(eps_sb[:], NORM_EPS)
            nw_sb = const.tile([128, KC], F32)
            for k in range(KC):
                nc.sync.dma_start(
                    out=nw_sb[:, k : k + 1], in_=nw_d[128 * k : 128 * (k + 1)]
                )
            # resident transposed ternary weight, chunk k at cols [k*DOUT, (k+1)*DOUT)
            wqt = const.tile([128, KC * DOUT], BF16)

            # ---- gamma: partial |w| sum over this core's slice + AllReduce ----
            asb = spool.tile([128, 2], F32)
            for i in range(WS // 128):
                wst = gpool.tile([128, DIN], F32)
                nc.sync.dma_start(
                    out=wst[:], in_=ws_d[128 * i : 128 * (i + 1), :]
                )
                nc.vector.tensor_reduce(
                    asb[:, i : i + 1],
                    wst[:],
                    axis=mybir.AxisListType.X,
                    op=ALU.add,
                    apply_absolute_value=True,
                )
            asum = spool.tile([128, 1], F32)
            nc.vector.tensor_tensor(asum[:], asb[:, 0:1], asb[:, 1:2], op=ALU.add)
            gps = psg.tile([128, 1], F32, tag="g")
            # ones.T @ asum -> slice total replicated on every partition
            nc.tensor.matmul(gps[:], ones[:], asum[:], start=True, stop=True)
            gpart = spool.tile([128, 1], F32)
            nc.scalar.copy(gpart[:], gps[:])

            gsum1 = spool.tile([1, CCPAD], F32)
            if os.environ.get("BITLINEAR_NO_CC"):
                # debug: skip the collective; gamma from this core's slice x8
                nc.vector.tensor_scalar(
                    gsum1[:, 0:1], gpart[0:1, :], float(NCORES), None,
                    op0=ALU.mult,
                )
            else:
                # pad the AllReduce payload (tiny transfers can stall the
                # collective firmware); only element [0,0] matters
                zpad = spool.tile([1, CCPAD], F32)
                nc.gpsimd.memset(zpad[:], 0.0)
                nc.vector.tensor_copy(zpad[0:1, 0:1], gpart[0:1, 0:1])
                ar_in = dramp.tile([1, CCPAD], F32)
                ar_out = dramp.tile([1, CCPAD], F32, addr_space="Shared")
                nc.sync.dma_start(out=ar_in[:], in_=zpad[:])
                nc.gpsimd.collective_compute(
                    "AllReduce",
                    ALU.add,
                    replica_groups=[list(range(NCORES))],
                    ins=[ar_in.opt()],
                    outs=[ar_out.opt()],
                )
                nc.sync.dma_start(out=gsum1[:], in_=ar_out[:])
            # broadcast partition 0 to all 128 partitions via K=1 matmul
            gbps = psg.tile([128, 1], F32, tag="g")
            nc.tensor.matmul(
                gbps[:], ones[0:1, :], gsum1[0:1, 0:1], start=True, stop=True
            )
            gamma = spool.tile([128, 1], F32)
            nc.vector.tensor_scalar(
                gamma[:], gbps[:], 1.0 / (DOUT * DIN), None, op0=ALU.mult
            )
            tau = spool.tile([128, 1], F32)
            nc.vector.tensor_scalar(
                tau[:], gamma[:], QUANT_EPS, 0.5, op0=ALU.add, op1=ALU.mult
            )
            ntau = spool.tile([128, 1], F32)
            nc.vector.tensor_scalar(ntau[:], tau[:], -1.0, None, op0=ALU.mult)

            stage = os.environ.get("BITLINEAR_STAGE", "full")
            if stage == "gamma":
                dbg = osb.tile([128, 512], F32)
                nc.vector.tensor_copy(dbg[:, 0:1], tau[:])
                nc.sync.dma_start(out=out_d[0:128, 0:512], in_=dbg[:])

            # ---- x path: rmsnorm + cast + transpose (gain fused into copy) ----
            xnt = []
            for t in range(TT if stage not in ("gamma",) else 0):
                xt = xin.tile([128, DIN], F32)
                nc.sync.dma_start(out=xt[:], in_=x_d[128 * t : 128 * (t + 1), :])
                ss = spool.tile([128, 1], F32)
                xn = xnp.tile([128, DIN], BF16)
                # scratch out (overwritten below); accum_out = sum(x*x)
                # (InstTensorTensorReduce crashes this HW path; ACT Square
                # with accum_out does the same job)
                nc.scalar.activation(
                    xn[:], xt[:], ACTF.Square, accum_out=ss[:]
                )
                rms = spool.tile([128, 1], F32)
                nc.scalar.activation(
                    rms[:], ss[:], ACTF.Sqrt, bias=eps_sb[:], scale=1.0 / DIN
                )
                rinv = spool.tile([128, 1], F32)
                nc.vector.reciprocal(rinv[:], rms[:])
                nc.vector.tensor_scalar(xn[:], xt[:], rinv[:], None, op0=ALU.mult)
                xx = xntp.tile([128, KC * 128], BF16)
                xnt.append(xx)
                if stage == "x1":
                    nc.vector.tensor_copy(xx[:], xn[:])
                    continue
                for k in range(KC):
                    pt = pst.tile([128, 128], BF16)
                    nc.tensor.transpose(
                        pt[:], xn[:, 128 * k : 128 * (k + 1)], ident[:]
                    )
                    dst = xx[:, 128 * k : 128 * (k + 1)]
                    if k % 2 == 0:
                        nc.vector.tensor_scalar(
                            dst, pt[:], nw_sb[:, k : k + 1], None, op0=ALU.mult
                        )
                    else:
                        nc.scalar.mul(dst, pt[:], nw_sb[:, k : k + 1])

            if stage in ("x", "x1"):
                for t in range(TT):
                    for n in range(NB):
                        ob = osb.tile([128, 512], F32)
                        nc.vector.tensor_copy(
                            ob[:], xnt[t][:, 512 * n : 512 * (n + 1)]
                        )
                        nc.sync.dma_start(
                            out=out_d[
                                128 * t : 128 * (t + 1), 512 * n : 512 * (n + 1)
                            ],
                            in_=ob[:],
                        )

            # ---- weight quantize+transpose and matmuls, n-block major ----
            for n in range(NB if stage in ("full", "w") else 0):
                for d4 in range(4):
                    d = 4 * n + d4
                    wt = wf.tile([128, DIN], F32)
                    nc.sync.dma_start(
                        out=wt[:], in_=w_d[128 * d : 128 * (d + 1), :]
                    )
                    neg = wm.tile([128, DIN], BF16)
                    nc.gpsimd.tensor_scalar(
                        neg[:], wt[:], ntau[:], None, op0=ALU.is_le
                    )
                    wq = wqp.tile([128, DIN], BF16)
                    # (w >= tau) - (w <= -tau)  in {-1, 0, +1}
                    nc.vector.scalar_tensor_tensor(
                        out=wq[:],
                        in0=wt[:],
                        scalar=tau[:],
                        in1=neg[:],
                        op0=ALU.is_ge,
                        op1=ALU.subtract,
                    )
                    for k in range(KC):
                        pt = pst.tile([128, 128], BF16)
                        nc.tensor.transpose(
                            pt[:], wq[:, 128 * k : 128 * (k + 1)], ident[:]
                        )
                        dst = wqt[:, k * DOUT + 128 * d : k * DOUT + 128 * (d + 1)]
                        if k % 2 == 0:
                            nc.vector.tensor_copy(dst, pt[:])
                        else:
                            nc.scalar.copy(dst, pt[:])
                for t in range(TT if stage == "full" else 0):
                    po = pso.tile([128, 512], F32)
                    for k in range(KC):
                        nc.tensor.matmul(
                            po[:],
                            xnt[t][:, 128 * k : 128 * (k + 1)],
                            wqt[:, k * DOUT + 512 * n : k * DOUT + 512 * (n + 1)],
                            start=(k == 0),
                            stop=(k == KC - 1),
                        )
                    ob = osb.tile([128, 512], F32)
                    nc.scalar.mul(ob[:], po[:], gamma[:])
                    nc.sync.dma_start(
                        out=out_d[
                            128 * t : 128 * (t + 1), 512 * n : 512 * (n + 1)
                        ],
                        in_=ob[:],
                    )

    nc.compile()
    return nc


_cached_nc = None


def _run_traced(nc, in_maps):
    """Execute with NTFF profiling, tolerating XLA's duplicate _body
    executables (keep only the newest NTFF before conversion)."""
    import glob
    import shutil
    import tempfile

    import antenv.axon_hooks as ah
    import gauge.profiler
    from concourse import bass_utils as bu

    core_ids = list(range(NCORES))
    neff_dir = os.environ.get("BASS_KERNEL_TRACE_DIR") or tempfile.mkdtemp(
        prefix="bitlinear_prof_"
    )
    shutil.rmtree(neff_dir, ignore_errors=True)
    os.makedirs(neff_dir, exist_ok=True)

    hook = ah.get_axon_ntff_profile_hook()
    with hook(neff_dir, [0]):
        res = run_bass_kernel_spmd(nc, in_maps, core_ids=core_ids)

    ntffs = sorted(
        glob.glob(os.path.join(neff_dir, "*_body*.ntff")), key=os.path.getmtime
    )
    if not ntffs:
        print("HW exec time: unavailable (no NTFF produced)")
        return res
    for f in ntffs[:-1]:
        os.remove(f)
    profile = gauge.profiler.Profile(
        profile_path=bu.FishPath(neff_dir),
        kernel_dev_mode=True,
        profile_on_exit=False,
        bass_kernel=nc.m,
        offline_processing=True,
        fname="*_body*",
        metadata={},
    )
    pr = bu._process_ntff_profile(
        profile, neff_dir, nc, core_ids, None, False, {}, trace_events=False
    )
    if pr.exec_time_ns is not None:
        print(f"HW exec time: {pr.exec_time_ns} ns")
    return pr.as_bass_kernel_results(res.results)


def kernel(x, weight, norm_weight):
    global _cached_nc
    if _cached_nc is None:
        _cached_nc = _build()
    nc = _cached_nc

    xf = np.ascontiguousarray(
        np.asarray(x, dtype=np.float32).reshape(TOKS, DIN)
    )
    w = np.ascontiguousarray(np.asarray(weight, dtype=np.float32))
    nw = np.ascontiguousarray(np.asarray(norm_weight, dtype=np.float32))

    in_maps = []
    for c in range(NCORES):
        in_maps.append(
            {
                "x": xf[TOK * c : TOK * (c + 1)],
                "weight": w,
                "wslice": np.ascontiguousarray(w[WS * c : WS * (c + 1)]),
                "norm_weight": nw,
            }
        )

    trace = bool(os.environ.get("BASS_KERNEL_TRACE"))
    if trace:
        res = _run_traced(nc, in_maps)
    else:
        res = run_bass_kernel_spmd(nc, in_maps, core_ids=list(range(NCORES)))
    outs = [np.asarray(res.results[c]["out"]) for c in range(NCORES)]
    return np.concatenate(outs, axis=0).reshape(B, S, DOUT).astype(np.float32)


# revision 20
# speedup vs baseline: 1.6841x; 1.6841x over previous
"""BitLinear (RMSNorm + ternary-quantized linear) on 8 TRN2 NeuronCores.

Sharding: data-parallel over tokens (B*S = 8192 -> 1024 per core), weight
replicated. gamma = mean(|w|) is computed exactly via per-core partial sums
over a 256-row slice of the weight (passed as the extra sharded input
"wslice") + a tiny AllReduce, so all cores quantize with the identical gamma.

Math per core:
  xn   = x / sqrt(mean(x^2) + 1e-6) * norm_weight        (f32 stats, bf16 out)
  w_q  = sign(w) * (|w| >= 0.5*(gamma + 1e-8))           ({-1,0,1}, exact bf16)
  out  = (xn @ w_q^T) * gamma                            (bf16 matmul, f32 out)

The threshold form equals clip(round(w/(gamma+eps)), -1, 1) because
max|w| < 1.5*gamma for this weight distribution (and values rounding to +-2
clip back to +-1 anyway).
"""

import os
import sys

for _p in ("/opt/trn_rl_repo",):
    if _p not in sys.path:
        sys.path.insert(0, _p)

import numpy as np

import concourse.bass as bass
import concourse.bacc as bacc
import concourse.tile as tile
import concourse.mybir as mybir
from concourse import masks
from concourse.bass_utils import run_bass_kernel_spmd

NORM_EPS = 1e-6
QUANT_EPS = 1e-8

B, S, DIN, DOUT = 2, 4096, 2048, 2048
NCORES = 8
TOKS = B * S              # 8192 total tokens
TOK = TOKS // NCORES      # 1024 tokens per core
TT = TOK // 128           # 8 token tiles per core
KC = DIN // 128           # 16 contraction chunks
NB = DOUT // 512          # 4 output column blocks
WS = DOUT // NCORES       # 256 weight rows per core for the gamma partial
CCPAD = 512               # AllReduce payload padded to 2KB

F32 = mybir.dt.float32
BF16 = mybir.dt.bfloat16
ALU = mybir.AluOpType
ACTF = mybir.ActivationFunctionType


def _build():
    nc = bacc.Bacc(
        "TRN2", target_bir_lowering=False, debug=False, num_devices=NCORES
    )

    x_d = nc.dram_tensor("x", [TOK, DIN], F32, kind="ExternalInput")
    w_d = nc.dram_tensor("weight", [DOUT, DIN], F32, kind="ExternalInput")
    ws_d = nc.dram_tensor("wslice", [WS, DIN], F32, kind="ExternalInput")
    nw_d = nc.dram_tensor("norm_weight", [DIN], F32, kind="ExternalInput")
    out_d = nc.dram_tensor("out", [TOK, DOUT], F32, kind="ExternalOutput")

    with tile.TileContext(nc) as tc:
        with (
            tc.tile_pool(name="const", bufs=1) as const,
            tc.tile_pool(name="dramp", bufs=1, space="DRAM") as dramp,
            tc.tile_pool(name="gpool", bufs=1) as gpool,
            tc.tile_pool(name="spool", bufs=4) as spool,
            tc.tile_pool(name="xin", bufs=2) as xin,
            tc.tile_pool(name="xnp", bufs=2) as xnp,
            tc.tile_pool(name="xntp", bufs=TT) as xntp,
            tc.tile_pool(name="wf", bufs=3) as wf,
            tc.tile_pool(name="wm", bufs=2) as wm,
            tc.tile_pool(name="wqp", bufs=2) as wqp,
            tc.tile_pool(name="osb", bufs=3) as osb,
            tc.tile_pool(name="psg", bufs=1, space="PSUM") as psg,
            tc.tile_pool(name="pst", bufs=4, space="PSUM") as pst,
            tc.tile_pool(name="pso", bufs=3, space="PSUM") as pso,
        ):
            # ---- constants ----
            ident = const.tile([128, 128], BF16)
            masks.make_identity(nc, ident[:])
            ones = const.tile([128, 128], F32)
            nc.gpsimd.memset(ones[:], 1.0)
            eps_sb = const.tile([128, 1], F32)
            nc.gpsimd.memset(eps_sb[:], NORM_EPS)
            nw_sb = const.tile([128, KC], F32)
            for k in range(KC):
                nc.sync.dma_start(
                    out=nw_sb[:, k : k + 1], in_=nw_d[128 * k : 128 * (k + 1)]
                )
            # resident transposed ternary weight, chunk k at cols [k*DOUT, (k+1)*DOUT)
            wqt = const.tile([128, KC * DOUT], BF16)

            # ---- gamma: partial |w| sum over this core's slice + AllReduce ----
            asb = spool.tile([128, 2], F32)
            for i in range(WS // 128):
                wst = gpool.tile([128, DIN], F32)
                nc.sync.dma_start(
                    out=wst[:], in_=ws_d[128 * i : 128 * (i + 1), :]
                )
                nc.vector.tensor_reduce(
                    asb[:, i : i + 1],
                    wst[:],
                    axis=mybir.AxisListType.X,
                    op=ALU.add,
                    apply_absolute_value=True,
                )
            asum = spool.tile([128, 1], F32)
            nc.vector.tensor_tensor(asum[:], asb[:, 0:1], asb[:, 1:2], op=ALU.add)
            gps = psg.tile([128, 1], F32, tag="g")
            # ones.T @ asum -> slice total replicated on every partition
            nc.tensor.matmul(gps[:], ones[:], asum[:], start=True, stop=True)
            gpart = spool.tile([128, 1], F32)
            nc.scalar.copy(gpart[:], gps[:])

            gsum1 = spool.tile([1, CCPAD], F32)
            if os.environ.get("BITLINEAR_NO_CC"):
                # debug: skip the collective; gamma from this core's slice x8
                nc.vector.tensor_scalar(
                    gsum1[:, 0:1], gpart[0:1, :], float(NCORES), None,
                    op0=ALU.mult,
                )
            else:
                # pad the AllReduce payload (tiny transfers can stall the
                # collective firmware); only element [0,0] matters
                zpad = spool.tile([1, CCPAD], F32)
                nc.gpsimd.memset(zpad[:], 0.0)
                nc.vector.tensor_copy(zpad[0:1, 0:1], gpart[0:1, 0:1])
                ar_in = dramp.tile([1, CCPAD], F32)
                ar_out = dramp.tile([1, CCPAD], F32, addr_space="Shared")
                nc.sync.dma_start(out=ar_in[:], in_=zpad[:])
                nc.gpsimd.collective_compute(
                    "AllReduce",
                    ALU.add,
                    replica_groups=[list(range(NCORES))],
                    ins=[ar_in.opt()],
                    outs=[ar_out.opt()],
                )
                nc.sync.dma_start(out=gsum1[:], in_=ar_out[:])
            # broadcast partition 0 to all 128 partitions via K=1 matmul
            gbps = psg.tile([128, 1], F32, tag="g")
            nc.tensor.matmul(
                gbps[:], ones[0:1, :], gsum1[0:1, 0:1], start=True, stop=True
            )
            gamma = spool.tile([128, 1], F32)
            nc.vector.tensor_scalar(
                gamma[:], gbps[:], 1.0 / (DOUT * DIN), None, op0=ALU.mult
            )
            tau = spool.tile([128, 1], F32)
            nc.vector.tensor_scalar(
                tau[:], gamma[:], QUANT_EPS, 0.5, op0=ALU.add, op1=ALU.mult
            )
            ntau = spool.tile([128, 1], F32)
            nc.vector.tensor_scalar(ntau[:], tau[:], -1.0, None, op0=ALU.mult)

            stage = os.environ.get("BITLINEAR_STAGE", "full")
            if stage == "gamma":
                dbg = osb.tile([128, 512], F32)
                nc.vector.tensor_copy(dbg[:, 0:1], tau[:])
                nc.sync.dma_start(out=out_d[0:128, 0:512], in_=dbg[:])

            # ---- x path: rmsnorm + cast + transpose (gain fused into copy) ----
            xnt = []
            for t in range(TT if stage not in ("gamma",) else 0):
                xt = xin.tile([128, DIN], F32)
                nc.sync.dma_start(out=xt[:], in_=x_d[128 * t : 128 * (t + 1), :])
                ss = spool.tile([128, 1], F32)
                xn = xnp.tile([128, DIN], BF16)
                # scratch out (overwritten below); accum_out = sum(x*x)
                # (InstTensorTensorReduce crashes this HW path; ACT Square
                # with accum_out does the same job)
                nc.scalar.activation(
                    xn[:], xt[:], ACTF.Square, accum_out=ss[:]
                )
                rms = spool.tile([128, 1], F32)
                nc.scalar.activation(
                    rms[:], ss[:], ACTF.Sqrt, bias=eps_sb[:], scale=1.0 / DIN
                )
                rinv = spool.tile([128, 1], F32)
                nc.vector.reciprocal(rinv[:], rms[:])
                nc.vector.tensor_scalar(xn[:], xt[:], rinv[:], None, op0=ALU.mult)
                xx = xntp.tile([128, KC * 128], BF16)
                xnt.append(xx)
                if stage == "x1":
                    nc.vector.tensor_copy(xx[:], xn[:])
                    continue
                for k in range(KC):
                    pt = pst.tile([128, 128], BF16)
                    nc.tensor.transpose(
                        pt[:], xn[:, 128 * k : 128 * (k + 1)], ident[:]
                    )
                    dst = xx[:, 128 * k : 128 * (k + 1)]
                    if k % 2 == 0:
                        nc.vector.tensor_scalar(
                            dst, pt[:], nw_sb[:, k : k + 1], None, op0=ALU.mult
                        )
                    else:
                        nc.scalar.mul(dst, pt[:], nw_sb[:, k : k + 1])

            if stage in ("x", "x1"):
                for t in range(TT):
                    for n in range(NB):
                        ob = osb.tile([128, 512], F32)
                        nc.vector.tensor_copy(
                            ob[:], xnt[t][:, 512 * n : 512 * (n + 1)]
                        )
                        nc.sync.dma_start(
                            out=out_d[
                                128 * t : 128 * (t + 1), 512 * n : 512 * (n + 1)
                            ],
                            in_=ob[:],
                        )

            # ---- weight quantize+transpose and matmuls, n-block major ----
            for n in range(NB if stage in ("full", "w") else 0):
                for d4 in range(4):
                    d = 4 * n + d4
                    wt = wf.tile([128, DIN], F32)
                    nc.sync.dma_start(
                        out=wt[:], in_=w_d[128 * d : 128 * (d + 1), :]
                    )
                    # w_q = (w >= tau) - (w <= -tau), three single-op DVE
                    # passes (gpsimd tensor_scalar / DVE scalar_tensor_tensor
                    # measure 24-31us per tile on this HW; fused two-op
                    # tensor_scalar with an AP scalar fails ISA checks)
                    pos = wm.tile([128, DIN], BF16, tag="pos")
                    nc.vector.tensor_scalar(
                        pos[:], wt[:], tau[:], None, op0=ALU.is_ge
                    )
                    neg = wm.tile([128, DIN], BF16, tag="neg")
                    nc.vector.tensor_scalar(
                        neg[:], wt[:], ntau[:], None, op0=ALU.is_le
                    )
                    wq = wqp.tile([128, DIN], BF16)
                    nc.vector.tensor_tensor(wq[:], pos[:], neg[:], op=ALU.subtract)
                    for k in range(KC):
                        pt = pst.tile([128, 128], BF16)
                        nc.tensor.transpose(
                            pt[:], wq[:, 128 * k : 128 * (k + 1)], ident[:]
                        )
                        dst = wqt[:, k * DOUT + 128 * d : k * DOUT + 128 * (d + 1)]
                        if k % 2 == 0:
                            nc.vector.tensor_copy(dst, pt[:])
                        else:
                            nc.scalar.copy(dst, pt[:])
                for t in range(TT if stage == "full" else 0):
                    po = pso.tile([128, 512], F32)
                    for k in range(KC):
                        nc.tensor.matmul(
                            po[:],
                            xnt[t][:, 128 * k : 128 * (k + 1)],
                            wqt[:, k * DOUT + 512 * n : k * DOUT + 512 * (n + 1)],
                            start=(k == 0),
                            stop=(k == KC - 1),
                        )
                    ob = osb.tile([128, 512], F32)
                    nc.scalar.mul(ob[:], po[:], gamma[:])
                    nc.sync.dma_start(
                        out=out_d[
                            128 * t : 128 * (t + 1), 512 * n : 512 * (n + 1)
                        ],
                        in_=ob[:],
                    )

    nc.compile()
    return nc


_cached_nc = None


def _run_traced(nc, in_maps):
    """Execute with NTFF profiling, tolerating XLA's duplicate _body
    executables (keep only the newest NTFF before conversion)."""
    import glob
    import shutil
    import tempfile

    import antenv.axon_hooks as ah
    import gauge.profiler
    from concourse import bass_utils as bu

    core_ids = list(range(NCORES))
    neff_dir = os.environ.get("BASS_KERNEL_TRACE_DIR") or tempfile.mkdtemp(
        prefix="bitlinear_prof_"
    )
    shutil.rmtree(neff_dir, ignore_errors=True)
    os.makedirs(neff_dir, exist_ok=True)

    hook = ah.get_axon_ntff_profile_hook()
    with hook(neff_dir, [0]):
        res = run_bass_kernel_spmd(nc, in_maps, core_ids=core_ids)

    ntffs = sorted(
        glob.glob(os.path.join(neff_dir, "*_body*.ntff")), key=os.path.getmtime
    )
    if not ntffs:
        print("HW exec time: unavailable (no NTFF produced)")
        return res
    for f in ntffs[:-1]:
        os.remove(f)
    profile = gauge.profiler.Profile(
        profile_path=bu.FishPath(neff_dir),
        kernel_dev_mode=True,
        profile_on_exit=False,
        bass_kernel=nc.m,
        offline_processing=True,
        fname="*_body*",
        metadata={},
    )
    pr = bu._process_ntff_profile(
        profile, neff_dir, nc, core_ids, None, False, {}, trace_events=False
    )
    if pr.exec_time_ns is not None:
        print(f"HW exec time: {pr.exec_time_ns} ns")
    return pr.as_bass_kernel_results(res.results)


def kernel(x, weight, norm_weight):
    global _cached_nc
    if _cached_nc is None:
        _cached_nc = _build()
    nc = _cached_nc

    xf = np.ascontiguousarray(
        np.asarray(x, dtype=np.float32).reshape(TOKS, DIN)
    )
    w = np.ascontiguousarray(np.asarray(weight, dtype=np.float32))
    nw = np.ascontiguousarray(np.asarray(norm_weight, dtype=np.float32))

    in_maps = []
    for c in range(NCORES):
        in_maps.append(
            {
                "x": xf[TOK * c : TOK * (c + 1)],
                "weight": w,
                "wslice": np.ascontiguousarray(w[WS * c : WS * (c + 1)]),
                "norm_weight": nw,
            }
        )

    trace = bool(os.environ.get("BASS_KERNEL_TRACE"))
    if trace:
        res = _run_traced(nc, in_maps)
    else:
        res = run_bass_kernel_spmd(nc, in_maps, core_ids=list(range(NCORES)))
    outs = [np.asarray(res.results[c]["out"]) for c in range(NCORES)]
    return np.concatenate(outs, axis=0).reshape(B, S, DOUT).astype(np.float32)


# revision 22
# speedup vs baseline: 2.0576x; 1.2218x over previous
"""BitLinear (RMSNorm + ternary-quantized linear) on 8 TRN2 NeuronCores.

Sharding: data-parallel over tokens (B*S = 8192 -> 1024 per core), weight
replicated. gamma = mean(|w|) is computed locally on every core with a
first streaming pass over the full weight (abs row-sums + a ones-matmul
partition reduction). No collectives: an 8-core AllReduce measures ~150us
on this stack, far more than the extra 16MB weight re-read costs.

Math per core:
  xn   = x / sqrt(mean(x^2) + 1e-6) * norm_weight        (f32 stats, bf16 out)
  w_q  = (w >= tau) - (w <= -tau),  tau = 0.5*(gamma + 1e-8)   ({-1,0,+1})
  out  = (xn @ w_q^T) * gamma                            (bf16 matmul, f32 out)

The threshold form equals clip(round(w/(gamma+eps)), -1, 1) because
max|w| < 1.5*gamma for this weight distribution (and values rounding to +-2
clip back to +-1 anyway).

Engine notes from profiling this HW path:
  - gpsimd tensor_scalar and DVE scalar_tensor_tensor run 24-31us per
    [128,2048] tile -- avoid; single-op DVE tensor_scalar is ~1-2us.
  - InstTensorTensorReduce crashes the device; ACT Square+accum_out works.
  - Fused two-op tensor_scalar with an AP scalar in op1 fails ISA checks.
"""

import os
import sys

for _p in ("/opt/trn_rl_repo",):
    if _p not in sys.path:
        sys.path.insert(0, _p)

import numpy as np

import concourse.bass as bass
import concourse.bacc as bacc
import concourse.tile as tile
import concourse.mybir as mybir
from concourse import masks
from concourse.bass_utils import run_bass_kernel_spmd

NORM_EPS = 1e-6
QUANT_EPS = 1e-8

B, S, DIN, DOUT = 2, 4096, 2048, 2048
NCORES = 8
TOKS = B * S              # 8192 total tokens
TOK = TOKS // NCORES      # 1024 tokens per core
TT = TOK // 128           # 8 token tiles per core
KC = DIN // 128           # 16 contraction chunks
NB = DOUT // 512          # 4 output column blocks
WB = DOUT // 128          # 16 weight row blocks

F32 = mybir.dt.float32
BF16 = mybir.dt.bfloat16
ALU = mybir.AluOpType
ACTF = mybir.ActivationFunctionType


def _build():
    nc = bacc.Bacc(
        "TRN2", target_bir_lowering=False, debug=False, num_devices=NCORES
    )

    x_d = nc.dram_tensor("x", [TOK, DIN], F32, kind="ExternalInput")
    w_d = nc.dram_tensor("weight", [DOUT, DIN], F32, kind="ExternalInput")
    nw_d = nc.dram_tensor("norm_weight", [DIN], F32, kind="ExternalInput")
    out_d = nc.dram_tensor("out", [TOK, DOUT], F32, kind="ExternalOutput")

    with tile.TileContext(nc) as tc:
        with (
            tc.tile_pool(name="const", bufs=1) as const,
            tc.tile_pool(name="spool", bufs=4) as spool,
            tc.tile_pool(name="xin", bufs=2) as xin,
            tc.tile_pool(name="xnp", bufs=2) as xnp,
            tc.tile_pool(name="xntp", bufs=TT) as xntp,
            tc.tile_pool(name="wf", bufs=3) as wf,
            tc.tile_pool(name="wm", bufs=2) as wm,
            tc.tile_pool(name="wqp", bufs=2) as wqp,
            tc.tile_pool(name="osb", bufs=4) as osb,
            tc.tile_pool(name="pst", bufs=3, space="PSUM") as pst,
            tc.tile_pool(name="pso", bufs=1, space="PSUM") as pso,
        ):
            # ---- constants ----
            ident = const.tile([128, 128], BF16)
            masks.make_identity(nc, ident[:])
            ones = const.tile([128, 128], F32)
            nc.gpsimd.memset(ones[:], 1.0)
            eps_sb = const.tile([128, 1], F32)
            nc.gpsimd.memset(eps_sb[:], NORM_EPS)
            nw_sb = const.tile([128, KC], F32)
            for k in range(KC):
                nc.sync.dma_start(
                    out=nw_sb[:, k : k + 1], in_=nw_d[128 * k : 128 * (k + 1)]
                )
            # resident transposed ternary weight, chunk k at cols [k*DOUT, (k+1)*DOUT)
            wqt = const.tile([128, KC * DOUT], BF16)
            part = const.tile([128, WB], F32)

            # ---- pass 1: gamma = mean|w| over the full weight, locally ----
            for d in range(WB):
                wt = wf.tile([128, DIN], F32)
                nc.sync.dma_start(out=wt[:], in_=w_d[128 * d : 128 * (d + 1), :])
                if d % 2 == 0:
                    nc.vector.tensor_reduce(
                        part[:, d : d + 1],
                        wt[:],
                        axis=mybir.AxisListType.X,
                        op=ALU.add,
                        apply_absolute_value=True,
                    )
                else:
                    ascr = wm.tile([128, DIN], BF16, tag="ascr")
                    nc.scalar.activation(
                        ascr[:], wt[:], ACTF.Abs, accum_out=part[:, d : d + 1]
                    )
            asum = spool.tile([128, 1], F32)
            nc.vector.tensor_reduce(
                asum[:], part[:, :], axis=mybir.AxisListType.X, op=ALU.add
            )
            gps = pso.tile([128, 1], F32, tag="g", bufs=1)
            # ones.T @ asum -> total |w| sum replicated on every partition
            nc.tensor.matmul(gps[:], ones[:], asum[:], start=True, stop=True)
            gamma = spool.tile([128, 1], F32)
            nc.vector.tensor_scalar(
                gamma[:], gps[:], 1.0 / (DOUT * DIN), None, op0=ALU.mult
            )
            tau = spool.tile([128, 1], F32)
            nc.vector.tensor_scalar(
                tau[:], gamma[:], QUANT_EPS, 0.5, op0=ALU.add, op1=ALU.mult
            )
            ntau = spool.tile([128, 1], F32)
            nc.vector.tensor_scalar(ntau[:], tau[:], -1.0, None, op0=ALU.mult)

            # ---- x path: rmsnorm + cast + transpose (gain fused into copy) ----
            xnt = []
            for t in range(TT):
                xt = xin.tile([128, DIN], F32)
                nc.sync.dma_start(out=xt[:], in_=x_d[128 * t : 128 * (t + 1), :])
                ss = spool.tile([128, 1], F32)
                xn = xnp.tile([128, DIN], BF16)
                # xn is scratch here (overwritten below); accum_out = sum(x*x)
                nc.scalar.activation(xn[:], xt[:], ACTF.Square, accum_out=ss[:])
                rms = spool.tile([128, 1], F32)
                nc.scalar.activation(
                    rms[:], ss[:], ACTF.Sqrt, bias=eps_sb[:], scale=1.0 / DIN
                )
                rinv = spool.tile([128, 1], F32)
                nc.vector.reciprocal(rinv[:], rms[:])
                nc.vector.tensor_scalar(xn[:], xt[:], rinv[:], None, op0=ALU.mult)
                xx = xntp.tile([128, KC * 128], BF16)
                xnt.append(xx)
                for k in range(KC):
                    pt = pst.tile([128, 128], BF16)
                    nc.tensor.transpose(
                        pt[:], xn[:, 128 * k : 128 * (k + 1)], ident[:]
                    )
                    dst = xx[:, 128 * k : 128 * (k + 1)]
                    if k % 2 == 0:
                        nc.vector.tensor_scalar(
                            dst, pt[:], nw_sb[:, k : k + 1], None, op0=ALU.mult
                        )
                    else:
                        nc.scalar.mul(dst, pt[:], nw_sb[:, k : k + 1])

            # ---- pass 2: quantize + transpose the full weight ----
            for d in range(WB):
                wt = wf.tile([128, DIN], F32)
                nc.sync.dma_start(out=wt[:], in_=w_d[128 * d : 128 * (d + 1), :])
                pos = wm.tile([128, DIN], BF16, tag="pos")
                nc.vector.tensor_scalar(pos[:], wt[:], tau[:], None, op0=ALU.is_ge)
                neg = wm.tile([128, DIN], BF16, tag="neg")
                nc.vector.tensor_scalar(neg[:], wt[:], ntau[:], None, op0=ALU.is_le)
                wq = wqp.tile([128, DIN], BF16)
                nc.vector.tensor_tensor(wq[:], pos[:], neg[:], op=ALU.subtract)
                for k in range(KC):
                    pt = pst.tile([128, 128], BF16)
                    nc.tensor.transpose(
                        pt[:], wq[:, 128 * k : 128 * (k + 1)], ident[:]
                    )
                    dst = wqt[:, k * DOUT + 128 * d : k * DOUT + 128 * (d + 1)]
                    if k % 2 == 0:
                        nc.vector.tensor_copy(dst, pt[:])
                    else:
                        nc.scalar.copy(dst, pt[:])

            # ---- matmuls: k-outer so each xnT chunk is loaded once ----
            for t in range(TT):
                po = [
                    pso.tile(
                        [128, 512], F32, tag=f"po{n}", bufs=1, name=f"po{n}_{t}"
                    )
                    for n in range(NB)
                ]
                for k in range(KC):
                    for n in range(NB):
                        nc.tensor.matmul(
                            po[n][:],
                            xnt[t][:, 128 * k : 128 * (k + 1)],
                            wqt[:, k * DOUT + 512 * n : k * DOUT + 512 * (n + 1)],
                            start=(k == 0),
                            stop=(k == KC - 1),
                        )
                for n in range(NB):
                    ob = osb.tile([128, 512], F32)
                    nc.scalar.mul(ob[:], po[n][:], gamma[:])
                    nc.sync.dma_start(
                        out=out_d[
                            128 * t : 128 * (t + 1), 512 * n : 512 * (n + 1)
                        ],
                        in_=ob[:],
                    )

    nc.compile()
    return nc


_cached_nc = None


def _run_traced(nc, in_maps):
    """Execute with NTFF profiling, tolerating XLA's duplicate _body
    executables (keep only the newest NTFF before conversion)."""
    import glob
    import shutil
    import tempfile

    import antenv.axon_hooks as ah
    import gauge.profiler
    from concourse import bass_utils as bu

    core_ids = list(range(NCORES))
    neff_dir = os.environ.get("BASS_KERNEL_TRACE_DIR") or tempfile.mkdtemp(
        prefix="bitlinear_prof_"
    )
    shutil.rmtree(neff_dir, ignore_errors=True)
    os.makedirs(neff_dir, exist_ok=True)

    hook = ah.get_axon_ntff_profile_hook()
    with hook(neff_dir, [0]):
        res = run_bass_kernel_spmd(nc, in_maps, core_ids=core_ids)

    ntffs = sorted(
        glob.glob(os.path.join(neff_dir, "*_body*.ntff")), key=os.path.getmtime
    )
    if not ntffs:
        print("HW exec time: unavailable (no NTFF produced)")
        return res
    for f in ntffs[:-1]:
        os.remove(f)
    profile = gauge.profiler.Profile(
        profile_path=bu.FishPath(neff_dir),
        kernel_dev_mode=True,
        profile_on_exit=False,
        bass_kernel=nc.m,
        offline_processing=True,
        fname="*_body*",
        metadata={},
    )
    pr = bu._process_ntff_profile(
        profile, neff_dir, nc, core_ids, None, False, {}, trace_events=False
    )
    if pr.exec_time_ns is not None:
        print(f"HW exec time: {pr.exec_time_ns} ns")
    return pr.as_bass_kernel_results(res.results)


def kernel(x, weight, norm_weight):
    global _cached_nc
    if _cached_nc is None:
        _cached_nc = _build()
    nc = _cached_nc

    xf = np.ascontiguousarray(
        np.asarray(x, dtype=np.float32).reshape(TOKS, DIN)
    )
    w = np.ascontiguousarray(np.asarray(weight, dtype=np.float32))
    nw = np.ascontiguousarray(np.asarray(norm_weight, dtype=np.float32))

    in_maps = []
    for c in range(NCORES):
        in_maps.append(
            {
                "x": xf[TOK * c : TOK * (c + 1)],
                "weight": w,
                "norm_weight": nw,
            }
        )

    trace = bool(os.environ.get("BASS_KERNEL_TRACE"))
    if trace:
        res = _run_traced(nc, in_maps)
    else:
        res = run_bass_kernel_spmd(nc, in_maps, core_ids=list(range(NCORES)))
    outs = [np.asarray(res.results[c]["out"]) for c in range(NCORES)]
    return np.concatenate(outs, axis=0).reshape(B, S, DOUT).astype(np.float32)


# revision 24
# speedup vs baseline: 2.1770x; 1.0580x over previous
"""BitLinear (RMSNorm + ternary-quantized linear) on 8 TRN2 NeuronCores.

Sharding: data-parallel over tokens (B*S = 8192 -> 1024 per core), weight
replicated. gamma = mean(|w|) is computed locally on every core with a
first streaming pass over the full weight (abs row-sums + a ones-matmul
partition reduction). No collectives: an 8-core AllReduce measures ~150us
on this stack, far more than the extra 16MB weight re-read costs.

Math per core:
  xn   = x / sqrt(mean(x^2) + 1e-6) * norm_weight        (f32 stats, bf16 out)
  w_q  = (w >= tau) - (w <= -tau),  tau = 0.5*(gamma + 1e-8)   ({-1,0,+1})
  out  = (xn @ w_q^T) * gamma                            (bf16 matmul, f32 out)

The threshold form equals clip(round(w/(gamma+eps)), -1, 1) because
max|w| < 1.5*gamma for this weight distribution (and values rounding to +-2
clip back to +-1 anyway).

Engine notes from profiling this HW path:
  - gpsimd tensor_scalar and DVE scalar_tensor_tensor run 24-31us per
    [128,2048] tile -- avoid; single-op DVE tensor_scalar is ~1-2us.
  - InstTensorTensorReduce crashes the device; ACT Square+accum_out works.
  - Fused two-op tensor_scalar with an AP scalar in op1 fails ISA checks.
"""

import os
import sys

for _p in ("/opt/trn_rl_repo",):
    if _p not in sys.path:
        sys.path.insert(0, _p)

import numpy as np

import concourse.bass as bass
import concourse.bacc as bacc
import concourse.tile as tile
import concourse.mybir as mybir
from concourse import masks
from concourse.bass_utils import run_bass_kernel_spmd

NORM_EPS = 1e-6
QUANT_EPS = 1e-8

B, S, DIN, DOUT = 2, 4096, 2048, 2048
NCORES = 8
TOKS = B * S              # 8192 total tokens
TOK = TOKS // NCORES      # 1024 tokens per core
TT = TOK // 128           # 8 token tiles per core
KC = DIN // 128           # 16 contraction chunks
NB = DOUT // 512          # 4 output column blocks
WB = DOUT // 128          # 16 weight row blocks

F32 = mybir.dt.float32
BF16 = mybir.dt.bfloat16
ALU = mybir.AluOpType
ACTF = mybir.ActivationFunctionType


def _build():
    nc = bacc.Bacc(
        "TRN2", target_bir_lowering=False, debug=False, num_devices=NCORES
    )

    x_d = nc.dram_tensor("x", [TOK, DIN], F32, kind="ExternalInput")
    w_d = nc.dram_tensor("weight", [DOUT, DIN], F32, kind="ExternalInput")
    nw_d = nc.dram_tensor("norm_weight", [DIN], F32, kind="ExternalInput")
    out_d = nc.dram_tensor("out", [TOK, DOUT], F32, kind="ExternalOutput")

    with tile.TileContext(nc) as tc:
        with (
            tc.tile_pool(name="const", bufs=1) as const,
            tc.tile_pool(name="spool", bufs=4) as spool,
            tc.tile_pool(name="xin", bufs=2) as xin,
            tc.tile_pool(name="xnp", bufs=2) as xnp,
            tc.tile_pool(name="xntp", bufs=TT) as xntp,
            tc.tile_pool(name="wf", bufs=3) as wf,
            tc.tile_pool(name="wm", bufs=2) as wm,
            tc.tile_pool(name="wqp", bufs=2) as wqp,
            tc.tile_pool(name="osb", bufs=4) as osb,
            tc.tile_pool(name="pst", bufs=3, space="PSUM") as pst,
            tc.tile_pool(name="pso", bufs=1, space="PSUM") as pso,
        ):
            # ---- constants ----
            ident = const.tile([128, 128], BF16)
            masks.make_identity(nc, ident[:])
            ones = const.tile([128, 128], F32)
            nc.gpsimd.memset(ones[:], 1.0)
            eps_sb = const.tile([128, 1], F32)
            nc.gpsimd.memset(eps_sb[:], NORM_EPS)
            nw_sb = const.tile([128, KC], F32)
            for k in range(KC):
                nc.sync.dma_start(
                    out=nw_sb[:, k : k + 1], in_=nw_d[128 * k : 128 * (k + 1)]
                )
            # resident transposed ternary weight, chunk k at cols [k*DOUT, (k+1)*DOUT)
            wqt = const.tile([128, KC * DOUT], BF16)
            part = const.tile([128, WB], F32)

            # ---- x path: rmsnorm + cast + transpose (gain fused into copy).
            # Emitted first so the PE instruction stream starts with the x
            # transposes instead of head-of-line blocking on gamma. ----
            xnt = []
            for t in range(TT):
                xt = xin.tile([128, DIN], F32)
                nc.sync.dma_start(out=xt[:], in_=x_d[128 * t : 128 * (t + 1), :])
                ss = spool.tile([128, 1], F32)
                xn = xnp.tile([128, DIN], BF16)
                # xn is scratch here (overwritten below); accum_out = sum(x*x)
                nc.scalar.activation(xn[:], xt[:], ACTF.Square, accum_out=ss[:])
                rms = spool.tile([128, 1], F32)
                nc.scalar.activation(
                    rms[:], ss[:], ACTF.Sqrt, bias=eps_sb[:], scale=1.0 / DIN
                )
                rinv = spool.tile([128, 1], F32)
                nc.vector.reciprocal(rinv[:], rms[:])
                nc.vector.tensor_scalar(xn[:], xt[:], rinv[:], None, op0=ALU.mult)
                xx = xntp.tile([128, KC * 128], BF16)
                xnt.append(xx)
                for k in range(KC):
                    pt = pst.tile([128, 128], BF16)
                    nc.tensor.transpose(
                        pt[:], xn[:, 128 * k : 128 * (k + 1)], ident[:]
                    )
                    dst = xx[:, 128 * k : 128 * (k + 1)]
                    if k % 2 == 0:
                        nc.vector.tensor_scalar(
                            dst, pt[:], nw_sb[:, k : k + 1], None, op0=ALU.mult
                        )
                    else:
                        nc.scalar.mul(dst, pt[:], nw_sb[:, k : k + 1])

            # ---- pass 1: gamma = mean|w| over the full weight, locally.
            # Weight DMAs ride the SWDGE (gpsimd) queue so they stream in
            # parallel with the x tiles on the sync HWDGE queue. ----
            for d in range(WB):
                wt = wf.tile([128, DIN], F32)
                nc.gpsimd.dma_start(
                    out=wt[:], in_=w_d[128 * d : 128 * (d + 1), :]
                )
                if d % 2 == 0:
                    nc.vector.tensor_reduce(
                        part[:, d : d + 1],
                        wt[:],
                        axis=mybir.AxisListType.X,
                        op=ALU.add,
                        apply_absolute_value=True,
                    )
                else:
                    ascr = wm.tile([128, DIN], BF16, tag="ascr")
                    nc.scalar.activation(
                        ascr[:], wt[:], ACTF.Abs, accum_out=part[:, d : d + 1]
                    )
            asum = spool.tile([128, 1], F32)
            nc.vector.tensor_reduce(
                asum[:], part[:, :], axis=mybir.AxisListType.X, op=ALU.add
            )
            gps = pso.tile([128, 1], F32, tag="g", bufs=1)
            # ones.T @ asum -> total |w| sum replicated on every partition
            nc.tensor.matmul(gps[:], ones[:], asum[:], start=True, stop=True)
            gamma = spool.tile([128, 1], F32)
            nc.vector.tensor_scalar(
                gamma[:], gps[:], 1.0 / (DOUT * DIN), None, op0=ALU.mult
            )
            tau = spool.tile([128, 1], F32)
            nc.vector.tensor_scalar(
                tau[:], gamma[:], QUANT_EPS, 0.5, op0=ALU.add, op1=ALU.mult
            )
            ntau = spool.tile([128, 1], F32)
            nc.vector.tensor_scalar(ntau[:], tau[:], -1.0, None, op0=ALU.mult)

            # ---- pass 2 + matmuls, in two halves: quantize+transpose the
            # d-blocks for output columns [1024*h, 1024*(h+1)), then run the
            # matmuls for those columns while the other half streams in.
            # k-outer so each xnT stationary load serves 2 matmuls. ----
            for h in range(2):
                for d in range(8 * h, 8 * (h + 1)):
                    wt = wf.tile([128, DIN], F32)
                    nc.gpsimd.dma_start(
                        out=wt[:], in_=w_d[128 * d : 128 * (d + 1), :]
                    )
                    pos = wm.tile([128, DIN], BF16, tag="pos")
                    nc.vector.tensor_scalar(
                        pos[:], wt[:], tau[:], None, op0=ALU.is_ge
                    )
                    neg = wm.tile([128, DIN], BF16, tag="neg")
                    nc.vector.tensor_scalar(
                        neg[:], wt[:], ntau[:], None, op0=ALU.is_le
                    )
                    wq = wqp.tile([128, DIN], BF16)
                    nc.vector.tensor_tensor(wq[:], pos[:], neg[:], op=ALU.subtract)
                    for k in range(KC):
                        pt = pst.tile([128, 128], BF16)
                        nc.tensor.transpose(
                            pt[:], wq[:, 128 * k : 128 * (k + 1)], ident[:]
                        )
                        dst = wqt[:, k * DOUT + 128 * d : k * DOUT + 128 * (d + 1)]
                        if k % 2 == 0:
                            nc.vector.tensor_copy(dst, pt[:])
                        else:
                            nc.scalar.copy(dst, pt[:])
                for t in range(TT):
                    po = [
                        pso.tile(
                            [128, 512], F32, tag=f"po{n}", bufs=1,
                            name=f"po{n}_{t}",
                        )
                        for n in (2 * h, 2 * h + 1)
                    ]
                    for k in range(KC):
                        for i, n in enumerate((2 * h, 2 * h + 1)):
                            nc.tensor.matmul(
                                po[i][:],
                                xnt[t][:, 128 * k : 128 * (k + 1)],
                                wqt[
                                    :,
                                    k * DOUT + 512 * n : k * DOUT + 512 * (n + 1),
                                ],
                                start=(k == 0),
                                stop=(k == KC - 1),
                            )
                    for i, n in enumerate((2 * h, 2 * h + 1)):
                        ob = osb.tile([128, 512], F32)
                        nc.scalar.mul(ob[:], po[i][:], gamma[:])
                        nc.sync.dma_start(
                            out=out_d[
                                128 * t : 128 * (t + 1), 512 * n : 512 * (n + 1)
                            ],
                            in_=ob[:],
                        )

    nc.compile()
    return nc


_cached_nc = None


def _run_traced(nc, in_maps):
    """Execute with NTFF profiling, tolerating XLA's duplicate _body
    executables (keep only the newest NTFF before conversion)."""
    import glob
    import shutil
    import tempfile

    import antenv.axon_hooks as ah
    import gauge.profiler
    from concourse import bass_utils as bu

    core_ids = list(range(NCORES))
    neff_dir = os.environ.get("BASS_KERNEL_TRACE_DIR") or tempfile.mkdtemp(
        prefix="bitlinear_prof_"
    )
    shutil.rmtree(neff_dir, ignore_errors=True)
    os.makedirs(neff_dir, exist_ok=True)

    hook = ah.get_axon_ntff_profile_hook()
    with hook(neff_dir, [0]):
        res = run_bass_kernel_spmd(nc, in_maps, core_ids=core_ids)

    ntffs = sorted(
        glob.glob(os.path.join(neff_dir, "*_body*.ntff")), key=os.path.getmtime
    )
    if not ntffs:
        print("HW exec time: unavailable (no NTFF produced)")
        return res
    for f in ntffs[:-1]:
        os.remove(f)
    profile = gauge.profiler.Profile(
        profile_path=bu.FishPath(neff_dir),
        kernel_dev_mode=True,
        profile_on_exit=False,
        bass_kernel=nc.m,
        offline_processing=True,
        fname="*_body*",
        metadata={},
    )
    pr = bu._process_ntff_profile(
        profile, neff_dir, nc, core_ids, None, False, {}, trace_events=False
    )
    if pr.exec_time_ns is not None:
        print(f"HW exec time: {pr.exec_time_ns} ns")
    return pr.as_bass_kernel_results(res.results)


def kernel(x, weight, norm_weight):
    global _cached_nc
    if _cached_nc is None:
        _cached_nc = _build()
    nc = _cached_nc

    xf = np.ascontiguousarray(
        np.asarray(x, dtype=np.float32).reshape(TOKS, DIN)
    )
    w = np.ascontiguousarray(np.asarray(weight, dtype=np.float32))
    nw = np.ascontiguousarray(np.asarray(norm_weight, dtype=np.float32))

    in_maps = []
    for c in range(NCORES):
        in_maps.append(
            {
                "x": xf[TOK * c : TOK * (c + 1)],
                "weight": w,
                "norm_weight": nw,
            }
        )

    trace = bool(os.environ.get("BASS_KERNEL_TRACE"))
    if trace:
        res = _run_traced(nc, in_maps)
    else:
        res = run_bass_kernel_spmd(nc, in_maps, core_ids=list(range(NCORES)))
    outs = [np.asarray(res.results[c]["out"]) for c in range(NCORES)]
    return np.concatenate(outs, axis=0).reshape(B, S, DOUT).astype(np.float32)
